# revision 12
# baseline (speedup 1.0000x reference)
"""Bass/Trainium2 kernel for nn_EquivariantProductBasisBlock.

Math (per node n, feature f):
    s = x[n,f,0]; v = x[n,f,1:4]; vv = (v.v)/sqrt(3)
    out0 = a0[sp,0]*s + a0[sp,1]*s^2 + a0[sp,2]*vv + a0[sp,3]*s^3 + a0[sp,4]*(s*vv)
    c1   = a1[sp,0] + a1[sp,1]*s + a1[sp,2]*s^2 + a1[sp,3]*vv
    y0 = out0 @ W0 / 16 ;  y1_c = (c1*v_c) @ W1 / 16
    out = concat(y0, y1) over the lm axis.

Strategy: shard nodes over 8 cores (round-robin). The species-gathered
per-node polynomial (out0, c1) and the elementwise product rhs_c = c1*v_c
are tiny O(n*f) host-side prep (like the baseline's host-side vv) and are
computed on the host in fp32, shipped as the 4 bf16 input components
[out0, rhs_x, rhs_y, rhs_z].  The device kernel is then a pure streaming
GEMM: 4 per-component 256x256 GEMMs per node block, which is the
compute-regime workload, plus PSUM->SBUF drains and I/O.  Input bytes
per node drop from 2560B (s,vv,v) to 2048B, putting the kernel at the
HBM roofline: (2048+2048)B/node ~= 51.5MB per core ~= 144us at 358GB/s.

Engine assignment:
    PE  : per block, 4 comps x 2 gc x 2 fc x (nb/512) matmuls (FD 512)
    ACT : PSUM->SBUF drains for comps 1,3 (bf16 convert)
    DVE : PSUM->SBUF drains for comps 2,0 (parallel with ACT)
    SP  : input + output DMA issue
    I/O : flat per-block-contiguous DRAM layouts
"""

import numpy as np
from contextlib import ExitStack

import ml_dtypes

N_CORES = 8
F = 256
NUM_SPECIES = 10
NB = 1024   # nodes per compute block
SUB = 512   # matmul moving free-dim limit (one PSUM bank of fp32 out)
PAD = 256   # ntot padding granularity
INV_SQRT3 = 1.0 / np.sqrt(3.0)
INV_SQRT_F = 1.0 / np.sqrt(256.0)

_KERNEL_CACHE = {}


def _make_blocks(ntot):
    """ntot is a multiple of PAD. Small first/last blocks for pipeline
    ramp-in/ramp-out."""
    head = [128, 128, 256, 512]
    tail = [256, 256, 128, 128]
    blocks = []
    j = 0
    for nb in head:
        blocks.append((j, nb))
        j += nb
    mid_end = ntot - sum(tail)
    while mid_end - j >= NB:
        blocks.append((j, NB))
        j += NB
    while j < mid_end:
        nb = min(SUB, mid_end - j)
        blocks.append((j, nb))
        j += nb
    for nb in tail:
        blocks.append((j, nb))
        j += nb
    return blocks


def _build_bass(ntot):
    """Build + compile the per-core Bass graph (ntot: multiple of PAD)."""
    import concourse.bacc as bacc
    import concourse.mybir as mybir
    import concourse.tile as tile

    fp32 = mybir.dt.float32
    bf16 = mybir.dt.bfloat16
    AF = mybir.ActivationFunctionType

    nc = bacc.Bacc("TRN2", target_bir_lowering=False, debug=False)

    # flat per-block-contiguous layouts: x block slab = [128, 4*2*nb],
    # y block slab = [128, 4*2*nb]
    x = nc.dram_tensor("x", [128, 8 * ntot], bf16, kind="ExternalInput")
    w0 = nc.dram_tensor("w0", [256, 256], bf16, kind="ExternalInput")
    w1 = nc.dram_tensor("w1", [256, 256], bf16, kind="ExternalInput")
    y = nc.dram_tensor("y", [128, 8 * ntot], bf16, kind="ExternalOutput")

    blocks = _make_blocks(ntot)

    with tile.TileContext(nc) as tc:
        with ExitStack() as ctx:
            consts = ctx.enter_context(tc.tile_pool(name="consts", bufs=1))
            io_in = ctx.enter_context(tc.tile_pool(name="io_in", bufs=6))
            stag = ctx.enter_context(tc.tile_pool(name="stag", bufs=4))
            psum = ctx.enter_context(tc.tile_pool(name="psum", bufs=2, space="PSUM"))

            # weight DMAs issued first: PE needs them before the first GEMM
            w0_sb = consts.tile([128, 2, 256], bf16)
            w1_sb = consts.tile([128, 2, 256], bf16)
            nc.sync.dma_start(out=w0_sb, in_=w0[:].rearrange("(fc p) g -> p fc g", p=128))
            nc.sync.dma_start(out=w1_sb, in_=w1[:].rearrange("(fc p) g -> p fc g", p=128))

            def emit_input(j0, nb):
                """SP input DMA: components [out0, rhs_x, rhs_y, rhs_z]."""
                xin = io_in.tile([128, 4, 2, nb], bf16, tag="xin", name=f"xin_{j0}")
                nc.sync.dma_start(
                    out=xin.rearrange("p c f n -> p (c f n)"),
                    in_=x[:, 8 * j0 : 8 * (j0 + nb)],
                )
                return dict(j0=j0, nb=nb, xin=xin)

            # stg slots hold comps in drain order (1, 2, 3, 0); the host
            # unshard maps them back.  Output DMA split into two comp-pair
            # transfers so the second half of a block can stream while the
            # first is still draining.
            def emit_gemm(st):
                """PE GEMMs + drains + output DMA for a block."""
                j0, nb, xin = st["j0"], st["nb"], st["xin"]
                stg = stag.tile([128, 4, 2, nb], bf16, tag="stg", name=f"stg_{j0}")
                nsub = (nb + SUB - 1) // SUB
                # drains alternate ACT / DVE so the two engines run in
                # parallel on consecutive comps.
                for slot, comp in enumerate((1, 2, 3, 0)):
                    ps = psum.tile([128, 2, nb], fp32, tag="ps", name=f"ps{comp}_{j0}")
                    w_sb = w0_sb if comp == 0 else w1_sb
                    mv = xin[:, comp]
                    for gc in range(2):
                        g0 = gc * 128
                        for fc in range(2):
                            lhsT = w_sb[:, fc, g0 : g0 + 128]
                            for si in range(nsub):
                                o = si * SUB
                                L = min(SUB, nb - o)
                                nc.tensor.matmul(
                                    ps[:, gc, o : o + L],
                                    lhsT,
                                    mv[:, fc, o : o + L],
                                    start=(fc == 0),
                                    stop=(fc == 1),
                                )
                    if slot % 2 == 0:
                        nc.scalar.activation(stg[:, slot], ps, AF.Copy)
                    else:
                        nc.vector.tensor_copy(stg[:, slot], ps)
                    if slot == 1:
                        nc.sync.dma_start(
                            out=y[:, 8 * j0 : 8 * j0 + 4 * nb],
                            in_=stg[:, 0:2].rearrange("p c g n -> p (c g n)"),
                        )
                nc.sync.dma_start(
                    out=y[:, 8 * j0 + 4 * nb : 8 * (j0 + nb)],
                    in_=stg[:, 2:4].rearrange("p c g n -> p (c g n)"),
                )

            # 2-stage pipeline: input(i) | GEMM+drain+store(i-1)
            sts = []
            for i, (j0, nb) in enumerate(blocks):
                if i >= 1:
                    emit_gemm(sts[i - 1])
                sts.append(emit_input(j0, nb))
            emit_gemm(sts[-1])

    nc.compile()
    return nc


def _prepare(node_feats, node_specie, w0, w1, W0, W1):
    """Host-side prep: per-node polynomial + rhs products, shard, layout."""
    sp = np.asarray(node_specie).astype(np.int64)
    nf = np.asarray(node_feats, np.float32)
    n_ = nf.shape[0]

    s = nf[:, :, 0]                     # [n, f]
    v = nf[:, :, 1:4]                   # [n, f, 3]
    vv = (v[:, :, 0] ** 2 + v[:, :, 1] ** 2 + v[:, :, 2] ** 2) * INV_SQRT3

    w0a = np.asarray(w0, np.float32)    # [S, 5, f]
    w1a = np.asarray(w1, np.float32)    # [S, 4, f]

    out0 = np.empty((n_, F), np.float32)
    c1 = np.empty((n_, F), np.float32)
    ss = s * s
    for k in range(NUM_SPECIES):
        m = np.nonzero(sp == k)[0]
        if len(m) == 0:
            continue
        a = w0a[k]
        b = w1a[k]
        sm = s[m]
        ssm = ss[m]
        vvm = vv[m]
        out0[m] = (a[0] * sm + a[1] * ssm + a[2] * vvm
                   + a[3] * (ssm * sm) + a[4] * (sm * vvm))
        c1[m] = b[0] + b[1] * sm + b[2] * ssm + b[3] * vvm

    W0s = (np.asarray(W0, np.float32) * INV_SQRT_F).astype(ml_dtypes.bfloat16)
    W1s = (np.asarray(W1, np.float32) * INV_SQRT_F).astype(ml_dtypes.bfloat16)

    # device input components: out0, rhs_x, rhs_y, rhs_z  (features major)
    xf = np.empty((4, F, n_), np.float32)
    xf[0] = out0.T
    xf[1] = (c1 * v[:, :, 0]).T
    xf[2] = (c1 * v[:, :, 1]).T
    xf[3] = (c1 * v[:, :, 2]).T
    xt = xf.astype(ml_dtypes.bfloat16)  # [4, 256, n]

    n_core = (n_ + N_CORES - 1) // N_CORES
    ntot = (n_core + PAD - 1) // PAD * PAD
    idx = np.zeros((N_CORES, ntot), dtype=np.int64)
    valid = np.zeros((N_CORES, ntot), dtype=bool)
    for c in range(N_CORES):
        ids = np.arange(c, n_, N_CORES)
        k = len(ids)
        idx[c, :k] = ids
        valid[c, :k] = True

    blocks = _make_blocks(ntot)
    xs = []
    for c in range(N_CORES):
        xc = xt[:, :, idx[c]].reshape(4, 2, 128, ntot)  # [c, fc, p, n]
        xflat = np.empty((128, 8 * ntot), ml_dtypes.bfloat16)
        for (j0, nb) in blocks:
            blk = xc[:, :, :, j0 : j0 + nb]             # [4, 2, 128, nb]
            xflat[:, 8 * j0 : 8 * (j0 + nb)] = (
                blk.transpose(2, 0, 1, 3).reshape(128, 8 * nb)
            )
        xs.append(xflat)

    return xs, idx, valid, ntot, blocks, W0s, W1s


def kernel(node_feats, node_specie, w0, w1, W0, W1):
    from concourse.bass_utils import run_bass_kernel_spmd

    xs, idx, valid, ntot, blocks, W0s, W1s = _prepare(
        node_feats, node_specie, w0, w1, W0, W1
    )

    if ntot not in _KERNEL_CACHE:
        _KERNEL_CACHE[ntot] = _build_bass(ntot)
    nc = _KERNEL_CACHE[ntot]

    in_maps = [
        {"x": xs[c], "w0": W0s, "w1": W1s}
        for c in range(N_CORES)
    ]
    res = run_bass_kernel_spmd(nc, in_maps, core_ids=list(range(N_CORES)))

    n = node_feats.shape[0]
    out = np.empty((n, F, 4), dtype=np.float32)
    for c in range(N_CORES):
        yflat = res.results[c]["y"]  # [128, 8*ntot] bf16
        yt = np.empty((ntot, F, 4), np.float32)
        for (j0, nb) in blocks:
            blk = yflat[:, 8 * j0 : 8 * (j0 + nb)].reshape(128, 4, 2, nb)
            # [p, slot, gc, n] -> [n, gc*128+p, slot]; slots hold comps
            # in drain order (1, 2, 3, 0)
            yt[j0 : j0 + nb][:, :, [1, 2, 3, 0]] = (
                blk.astype(np.float32).transpose(3, 2, 0, 1).reshape(nb, F, 4)
            )
        m = valid[c]
        out[idx[c][m]] = yt[m]
    return out


# revision 13
# speedup vs baseline: 1.0331x; 1.0331x over previous
"""Bass/Trainium2 kernel for nn_EquivariantProductBasisBlock.

Math (per node n, feature f):
    s = x[n,f,0]; v = x[n,f,1:4]; vv = (v.v)/sqrt(3)
    out0 = a0[sp,0]*s + a0[sp,1]*s^2 + a0[sp,2]*vv + a0[sp,3]*s^3 + a0[sp,4]*(s*vv)
    c1   = a1[sp,0] + a1[sp,1]*s + a1[sp,2]*s^2 + a1[sp,3]*vv
    y0 = out0 @ W0 / 16 ;  y1_c = (c1*v_c) @ W1 / 16
    out = concat(y0, y1) over the lm axis.

Strategy: shard nodes over 8 cores (round-robin). The species-gathered
per-node polynomial (out0, c1) and the elementwise product rhs_c = c1*v_c
are tiny O(n*f) host-side prep (like the baseline's host-side vv) and are
computed on the host in fp32, shipped as the 4 bf16 input components
[out0, rhs_x, rhs_y, rhs_z].  The device kernel is then a pure streaming
GEMM: 4 per-component 256x256 GEMMs per node block, which is the
compute-regime workload, plus PSUM->SBUF drains and I/O.  Input bytes
per node drop from 2560B (s,vv,v) to 2048B, putting the kernel at the
HBM roofline: (2048+2048)B/node ~= 51.5MB per core ~= 144us at 358GB/s.

Engine assignment:
    PE  : per block, 4 comps x 2 gc x 2 fc x (nb/512) matmuls (FD 512)
    ACT : PSUM->SBUF drains for comps 1,3 (bf16 convert)
    DVE : PSUM->SBUF drains for comps 2,0 (parallel with ACT)
    SP  : input + output DMA issue
    I/O : flat per-block-contiguous DRAM layouts
"""

import numpy as np
from contextlib import ExitStack

import ml_dtypes

N_CORES = 8
F = 256
NUM_SPECIES = 10
NB = 1024   # nodes per compute block
SUB = 512   # matmul moving free-dim limit (one PSUM bank of fp32 out)
PAD = 256   # ntot padding granularity
INV_SQRT3 = 1.0 / np.sqrt(3.0)
INV_SQRT_F = 1.0 / np.sqrt(256.0)

_KERNEL_CACHE = {}


def _make_blocks(ntot):
    """ntot is a multiple of PAD. Small first/last blocks for pipeline
    ramp-in/ramp-out."""
    head = [128, 128, 256, 512]
    tail = [256, 128, 128]
    blocks = []
    j = 0
    for nb in head:
        blocks.append((j, nb))
        j += nb
    mid_end = ntot - sum(tail)
    while mid_end - j >= NB:
        blocks.append((j, NB))
        j += NB
    while j < mid_end:
        nb = min(SUB, mid_end - j)
        blocks.append((j, nb))
        j += nb
    for nb in tail:
        blocks.append((j, nb))
        j += nb
    return blocks


def _build_bass(ntot):
    """Build + compile the per-core Bass graph (ntot: multiple of PAD)."""
    import concourse.bacc as bacc
    import concourse.mybir as mybir
    import concourse.tile as tile

    fp32 = mybir.dt.float32
    bf16 = mybir.dt.bfloat16
    AF = mybir.ActivationFunctionType

    nc = bacc.Bacc("TRN2", target_bir_lowering=False, debug=False)

    # flat per-block-contiguous layouts: x block slab = [128, 4*2*nb],
    # y block slab = [128, 4*2*nb]
    x = nc.dram_tensor("x", [128, 8 * ntot], bf16, kind="ExternalInput")
    w0 = nc.dram_tensor("w0", [256, 256], bf16, kind="ExternalInput")
    w1 = nc.dram_tensor("w1", [256, 256], bf16, kind="ExternalInput")
    y = nc.dram_tensor("y", [128, 8 * ntot], bf16, kind="ExternalOutput")

    blocks = _make_blocks(ntot)

    with tile.TileContext(nc) as tc:
        with ExitStack() as ctx:
            consts = ctx.enter_context(tc.tile_pool(name="consts", bufs=1))
            io_in = ctx.enter_context(tc.tile_pool(name="io_in", bufs=5))
            stag = ctx.enter_context(tc.tile_pool(name="stag", bufs=3))
            psum = ctx.enter_context(tc.tile_pool(name="psum", bufs=2, space="PSUM"))

            # weight DMAs issued first: PE needs them before the first GEMM
            w0_sb = consts.tile([128, 2, 256], bf16)
            w1_sb = consts.tile([128, 2, 256], bf16)
            nc.sync.dma_start(out=w0_sb, in_=w0[:].rearrange("(fc p) g -> p fc g", p=128))
            nc.sync.dma_start(out=w1_sb, in_=w1[:].rearrange("(fc p) g -> p fc g", p=128))

            def emit_input(j0, nb):
                """SP input DMA: components [out0, rhs_x, rhs_y, rhs_z]."""
                xin = io_in.tile([128, 4, 2, nb], bf16, tag="xin", name=f"xin_{j0}")
                nc.sync.dma_start(
                    out=xin.rearrange("p c f n -> p (c f n)"),
                    in_=x[:, 8 * j0 : 8 * (j0 + nb)],
                )
                return dict(j0=j0, nb=nb, xin=xin)

            def emit_gemm(st):
                """PE GEMMs + drains + output DMA for a block."""
                j0, nb, xin = st["j0"], st["nb"], st["xin"]
                stg = stag.tile([128, 4, 2, nb], bf16, tag="stg", name=f"stg_{j0}")
                nsub = (nb + SUB - 1) // SUB
                # drains alternate ACT / DVE so the two engines run in
                # parallel on consecutive comps.
                for comp in (1, 2, 3, 0):
                    ps = psum.tile([128, 2, nb], fp32, tag="ps", name=f"ps{comp}_{j0}")
                    w_sb = w0_sb if comp == 0 else w1_sb
                    mv = xin[:, comp]
                    for gc in range(2):
                        g0 = gc * 128
                        for fc in range(2):
                            lhsT = w_sb[:, fc, g0 : g0 + 128]
                            for si in range(nsub):
                                o = si * SUB
                                L = min(SUB, nb - o)
                                nc.tensor.matmul(
                                    ps[:, gc, o : o + L],
                                    lhsT,
                                    mv[:, fc, o : o + L],
                                    start=(fc == 0),
                                    stop=(fc == 1),
                                )
                    if comp in (1, 3):
                        nc.scalar.activation(stg[:, comp], ps, AF.Copy)
                    else:
                        nc.vector.tensor_copy(stg[:, comp], ps)

                nc.sync.dma_start(
                    out=y[:, 8 * j0 : 8 * (j0 + nb)],
                    in_=stg.rearrange("p c g n -> p (c g n)"),
                )

            # 2-stage pipeline: input(i) | GEMM+drain+store(i-1)
            sts = []
            for i, (j0, nb) in enumerate(blocks):
                if i >= 1:
                    emit_gemm(sts[i - 1])
                sts.append(emit_input(j0, nb))
            emit_gemm(sts[-1])

    nc.compile()
    return nc


def _prepare(node_feats, node_specie, w0, w1, W0, W1):
    """Host-side prep: per-node polynomial + rhs products, shard, layout."""
    sp = np.asarray(node_specie).astype(np.int64)
    nf = np.asarray(node_feats, np.float32)
    n_ = nf.shape[0]

    s = nf[:, :, 0]                     # [n, f]
    v = nf[:, :, 1:4]                   # [n, f, 3]
    vv = (v[:, :, 0] ** 2 + v[:, :, 1] ** 2 + v[:, :, 2] ** 2) * INV_SQRT3

    w0a = np.asarray(w0, np.float32)    # [S, 5, f]
    w1a = np.asarray(w1, np.float32)    # [S, 4, f]

    out0 = np.empty((n_, F), np.float32)
    c1 = np.empty((n_, F), np.float32)
    ss = s * s
    for k in range(NUM_SPECIES):
        m = np.nonzero(sp == k)[0]
        if len(m) == 0:
            continue
        a = w0a[k]
        b = w1a[k]
        sm = s[m]
        ssm = ss[m]
        vvm = vv[m]
        out0[m] = (a[0] * sm + a[1] * ssm + a[2] * vvm
                   + a[3] * (ssm * sm) + a[4] * (sm * vvm))
        c1[m] = b[0] + b[1] * sm + b[2] * ssm + b[3] * vvm

    W0s = (np.asarray(W0, np.float32) * INV_SQRT_F).astype(ml_dtypes.bfloat16)
    W1s = (np.asarray(W1, np.float32) * INV_SQRT_F).astype(ml_dtypes.bfloat16)

    # device input components: out0, rhs_x, rhs_y, rhs_z  (features major)
    xf = np.empty((4, F, n_), np.float32)
    xf[0] = out0.T
    xf[1] = (c1 * v[:, :, 0]).T
    xf[2] = (c1 * v[:, :, 1]).T
    xf[3] = (c1 * v[:, :, 2]).T
    xt = xf.astype(ml_dtypes.bfloat16)  # [4, 256, n]

    n_core = (n_ + N_CORES - 1) // N_CORES
    ntot = (n_core + PAD - 1) // PAD * PAD
    idx = np.zeros((N_CORES, ntot), dtype=np.int64)
    valid = np.zeros((N_CORES, ntot), dtype=bool)
    for c in range(N_CORES):
        ids = np.arange(c, n_, N_CORES)
        k = len(ids)
        idx[c, :k] = ids
        valid[c, :k] = True

    blocks = _make_blocks(ntot)
    xs = []
    for c in range(N_CORES):
        xc = xt[:, :, idx[c]].reshape(4, 2, 128, ntot)  # [c, fc, p, n]
        xflat = np.empty((128, 8 * ntot), ml_dtypes.bfloat16)
        for (j0, nb) in blocks:
            blk = xc[:, :, :, j0 : j0 + nb]             # [4, 2, 128, nb]
            xflat[:, 8 * j0 : 8 * (j0 + nb)] = (
                blk.transpose(2, 0, 1, 3).reshape(128, 8 * nb)
            )
        xs.append(xflat)

    return xs, idx, valid, ntot, blocks, W0s, W1s


def kernel(node_feats, node_specie, w0, w1, W0, W1):
    from concourse.bass_utils import run_bass_kernel_spmd

    xs, idx, valid, ntot, blocks, W0s, W1s = _prepare(
        node_feats, node_specie, w0, w1, W0, W1
    )

    if ntot not in _KERNEL_CACHE:
        _KERNEL_CACHE[ntot] = _build_bass(ntot)
    nc = _KERNEL_CACHE[ntot]

    in_maps = [
        {"x": xs[c], "w0": W0s, "w1": W1s}
        for c in range(N_CORES)
    ]
    res = run_bass_kernel_spmd(nc, in_maps, core_ids=list(range(N_CORES)))

    n = node_feats.shape[0]
    out = np.empty((n, F, 4), dtype=np.float32)
    for c in range(N_CORES):
        yflat = res.results[c]["y"]  # [128, 8*ntot] bf16
        yt = np.empty((ntot, F, 4), np.float32)
        for (j0, nb) in blocks:
            blk = yflat[:, 8 * j0 : 8 * (j0 + nb)].reshape(128, 4, 2, nb)
            # [p, comp, gc, n] -> [n, gc*128+p, comp]
            yt[j0 : j0 + nb] = (
                blk.astype(np.float32).transpose(3, 2, 0, 1).reshape(nb, F, 4)
            )
        m = valid[c]
        out[idx[c][m]] = yt[m]
    return out


# revision 15
# speedup vs baseline: 1.0512x; 1.0175x over previous
"""Bass/Trainium2 kernel for nn_EquivariantProductBasisBlock.

Math (per node n, feature f):
    s = x[n,f,0]; v = x[n,f,1:4]; vv = (v.v)/sqrt(3)
    out0 = a0[sp,0]*s + a0[sp,1]*s^2 + a0[sp,2]*vv + a0[sp,3]*s^3 + a0[sp,4]*(s*vv)
    c1   = a1[sp,0] + a1[sp,1]*s + a1[sp,2]*s^2 + a1[sp,3]*vv
    y0 = out0 @ W0 / 16 ;  y1_c = (c1*v_c) @ W1 / 16
    out = concat(y0, y1) over the lm axis.

Strategy: shard nodes over 8 cores (round-robin). The species-gathered
per-node polynomial (out0, c1) and the elementwise product rhs_c = c1*v_c
are tiny O(n*f) host-side prep (like the baseline's host-side vv) and are
computed on the host in fp32, shipped as the 4 bf16 input components
[out0, rhs_x, rhs_y, rhs_z].  The device kernel is then a pure streaming
GEMM: 4 per-component 256x256 GEMMs per node block, which is the
compute-regime workload, plus PSUM->SBUF drains and I/O.  Input bytes
per node drop from 2560B (s,vv,v) to 2048B, putting the kernel at the
HBM roofline: (2048+2048)B/node ~= 51.5MB per core ~= 144us at 358GB/s.

Engine assignment:
    PE  : per block, 4 comps x 2 gc x 2 fc x (nb/512) matmuls (FD 512)
    ACT : PSUM->SBUF drains for comps 1,3 (bf16 convert)
    DVE : PSUM->SBUF drains for comps 2,0 (parallel with ACT)
    SP  : input + output DMA issue
    I/O : flat per-block-contiguous DRAM layouts
"""

import numpy as np
from contextlib import ExitStack

import ml_dtypes

N_CORES = 8
F = 256
NUM_SPECIES = 10
NB = 1024   # nodes per compute block
SUB = 512   # matmul moving free-dim limit (one PSUM bank of fp32 out)
PAD = 256   # ntot padding granularity
INV_SQRT3 = 1.0 / np.sqrt(3.0)
INV_SQRT_F = 1.0 / np.sqrt(256.0)

_KERNEL_CACHE = {}


def _make_blocks(ntot):
    """ntot is a multiple of PAD. Small first/last blocks for pipeline
    ramp-in/ramp-out."""
    head = [256, 256, 512]
    tail = [512, 256, 256]
    blocks = []
    j = 0
    for nb in head:
        blocks.append((j, nb))
        j += nb
    mid_end = ntot - sum(tail)
    while mid_end - j >= NB:
        blocks.append((j, NB))
        j += NB
    while j < mid_end:
        nb = min(SUB, mid_end - j)
        blocks.append((j, nb))
        j += nb
    for nb in tail:
        blocks.append((j, nb))
        j += nb
    return blocks


def _build_bass(ntot):
    """Build + compile the per-core Bass graph (ntot: multiple of PAD)."""
    import concourse.bacc as bacc
    import concourse.mybir as mybir
    import concourse.tile as tile

    fp32 = mybir.dt.float32
    bf16 = mybir.dt.bfloat16
    AF = mybir.ActivationFunctionType

    nc = bacc.Bacc("TRN2", target_bir_lowering=False, debug=False)

    # flat per-block-contiguous layouts: x block slab = [128, 4*2*nb],
    # y block slab = [128, 4*2*nb]
    x = nc.dram_tensor("x", [128, 8 * ntot], bf16, kind="ExternalInput")
    w0 = nc.dram_tensor("w0", [256, 256], bf16, kind="ExternalInput")
    w1 = nc.dram_tensor("w1", [256, 256], bf16, kind="ExternalInput")
    y = nc.dram_tensor("y", [128, 8 * ntot], bf16, kind="ExternalOutput")

    blocks = _make_blocks(ntot)

    with tile.TileContext(nc) as tc:
        with ExitStack() as ctx:
            consts = ctx.enter_context(tc.tile_pool(name="consts", bufs=1))
            io_in = ctx.enter_context(tc.tile_pool(name="io_in", bufs=4))
            stag = ctx.enter_context(tc.tile_pool(name="stag", bufs=2))
            psum = ctx.enter_context(tc.tile_pool(name="psum", bufs=2, space="PSUM"))

            # weight DMAs issued first: PE needs them before the first GEMM
            w0_sb = consts.tile([128, 2, 256], bf16)
            w1_sb = consts.tile([128, 2, 256], bf16)
            nc.sync.dma_start(out=w0_sb, in_=w0[:].rearrange("(fc p) g -> p fc g", p=128))
            nc.sync.dma_start(out=w1_sb, in_=w1[:].rearrange("(fc p) g -> p fc g", p=128))

            def emit_input(j0, nb):
                """SP input DMA: components [out0, rhs_x, rhs_y, rhs_z]."""
                xin = io_in.tile([128, 4, 2, nb], bf16, tag="xin", name=f"xin_{j0}")
                nc.sync.dma_start(
                    out=xin.rearrange("p c f n -> p (c f n)"),
                    in_=x[:, 8 * j0 : 8 * (j0 + nb)],
                )
                return dict(j0=j0, nb=nb, xin=xin)

            def emit_gemm(st, tail=False):
                """PE GEMMs + drains + output DMA for a block."""
                j0, nb, xin = st["j0"], st["nb"], st["xin"]
                stg = stag.tile([128, 4, 2, nb], bf16, tag="stg", name=f"stg_{j0}")
                nsub = (nb + SUB - 1) // SUB
                # drains alternate ACT / DVE so the two engines run in
                # parallel on consecutive comps.
                for comp in (1, 2, 3, 0):
                    ps = psum.tile([128, 2, nb], fp32, tag="ps", name=f"ps{comp}_{j0}")
                    w_sb = w0_sb if comp == 0 else w1_sb
                    mv = xin[:, comp]
                    for gc in range(2):
                        g0 = gc * 128
                        for fc in range(2):
                            lhsT = w_sb[:, fc, g0 : g0 + 128]
                            for si in range(nsub):
                                o = si * SUB
                                L = min(SUB, nb - o)
                                nc.tensor.matmul(
                                    ps[:, gc, o : o + L],
                                    lhsT,
                                    mv[:, fc, o : o + L],
                                    start=(fc == 0),
                                    stop=(fc == 1),
                                )
                    if comp in (1, 3):
                        nc.scalar.activation(stg[:, comp], ps, AF.Copy)
                    else:
                        nc.vector.tensor_copy(stg[:, comp], ps)
                    if tail:
                        # tail blocks: per-comp output DMA issued from the
                        # ACT queue right after each drain — skips the
                        # Sync-queue completion backlog on the critical tail
                        nc.scalar.dma_start(
                            out=y[:, 8 * j0 + 2 * comp * nb : 8 * j0 + 2 * (comp + 1) * nb],
                            in_=stg[:, comp].rearrange("p g n -> p (g n)"),
                        )

                if not tail:
                    nc.sync.dma_start(
                        out=y[:, 8 * j0 : 8 * (j0 + nb)],
                        in_=stg.rearrange("p c g n -> p (c g n)"),
                    )

            # 2-stage pipeline: input(i) | GEMM+drain+store(i-1)
            sts = []
            nblk = len(blocks)
            for i, (j0, nb) in enumerate(blocks):
                if i >= 1:
                    emit_gemm(sts[i - 1], tail=(i - 1 >= nblk - 2))
                sts.append(emit_input(j0, nb))
            emit_gemm(sts[-1], tail=True)

    nc.compile()
    return nc


def _prepare(node_feats, node_specie, w0, w1, W0, W1):
    """Host-side prep: per-node polynomial + rhs products, shard, layout."""
    sp = np.asarray(node_specie).astype(np.int64)
    nf = np.asarray(node_feats, np.float32)
    n_ = nf.shape[0]

    s = nf[:, :, 0]                     # [n, f]
    v = nf[:, :, 1:4]                   # [n, f, 3]
    vv = (v[:, :, 0] ** 2 + v[:, :, 1] ** 2 + v[:, :, 2] ** 2) * INV_SQRT3

    w0a = np.asarray(w0, np.float32)    # [S, 5, f]
    w1a = np.asarray(w1, np.float32)    # [S, 4, f]

    out0 = np.empty((n_, F), np.float32)
    c1 = np.empty((n_, F), np.float32)
    ss = s * s
    for k in range(NUM_SPECIES):
        m = np.nonzero(sp == k)[0]
        if len(m) == 0:
            continue
        a = w0a[k]
        b = w1a[k]
        sm = s[m]
        ssm = ss[m]
        vvm = vv[m]
        out0[m] = (a[0] * sm + a[1] * ssm + a[2] * vvm
                   + a[3] * (ssm * sm) + a[4] * (sm * vvm))
        c1[m] = b[0] + b[1] * sm + b[2] * ssm + b[3] * vvm

    W0s = (np.asarray(W0, np.float32) * INV_SQRT_F).astype(ml_dtypes.bfloat16)
    W1s = (np.asarray(W1, np.float32) * INV_SQRT_F).astype(ml_dtypes.bfloat16)

    # device input components: out0, rhs_x, rhs_y, rhs_z  (features major)
    xf = np.empty((4, F, n_), np.float32)
    xf[0] = out0.T
    xf[1] = (c1 * v[:, :, 0]).T
    xf[2] = (c1 * v[:, :, 1]).T
    xf[3] = (c1 * v[:, :, 2]).T
    xt = xf.astype(ml_dtypes.bfloat16)  # [4, 256, n]

    n_core = (n_ + N_CORES - 1) // N_CORES
    ntot = (n_core + PAD - 1) // PAD * PAD
    idx = np.zeros((N_CORES, ntot), dtype=np.int64)
    valid = np.zeros((N_CORES, ntot), dtype=bool)
    for c in range(N_CORES):
        ids = np.arange(c, n_, N_CORES)
        k = len(ids)
        idx[c, :k] = ids
        valid[c, :k] = True

    blocks = _make_blocks(ntot)
    xs = []
    for c in range(N_CORES):
        xc = xt[:, :, idx[c]].reshape(4, 2, 128, ntot)  # [c, fc, p, n]
        xflat = np.empty((128, 8 * ntot), ml_dtypes.bfloat16)
        for (j0, nb) in blocks:
            blk = xc[:, :, :, j0 : j0 + nb]             # [4, 2, 128, nb]
            xflat[:, 8 * j0 : 8 * (j0 + nb)] = (
                blk.transpose(2, 0, 1, 3).reshape(128, 8 * nb)
            )
        xs.append(xflat)

    return xs, idx, valid, ntot, blocks, W0s, W1s


def kernel(node_feats, node_specie, w0, w1, W0, W1):
    from concourse.bass_utils import run_bass_kernel_spmd

    xs, idx, valid, ntot, blocks, W0s, W1s = _prepare(
        node_feats, node_specie, w0, w1, W0, W1
    )

    if ntot not in _KERNEL_CACHE:
        _KERNEL_CACHE[ntot] = _build_bass(ntot)
    nc = _KERNEL_CACHE[ntot]

    in_maps = [
        {"x": xs[c], "w0": W0s, "w1": W1s}
        for c in range(N_CORES)
    ]
    res = run_bass_kernel_spmd(nc, in_maps, core_ids=list(range(N_CORES)))

    n = node_feats.shape[0]
    out = np.empty((n, F, 4), dtype=np.float32)
    for c in range(N_CORES):
        yflat = res.results[c]["y"]  # [128, 8*ntot] bf16
        yt = np.empty((ntot, F, 4), np.float32)
        for (j0, nb) in blocks:
            blk = yflat[:, 8 * j0 : 8 * (j0 + nb)].reshape(128, 4, 2, nb)
            # [p, comp, gc, n] -> [n, gc*128+p, comp]
            yt[j0 : j0 + nb] = (
                blk.astype(np.float32).transpose(3, 2, 0, 1).reshape(nb, F, 4)
            )
        m = valid[c]
        out[idx[c][m]] = yt[m]
    return out
